# revision 24
# baseline (speedup 1.0000x reference)
"""DIEN forward kernel for Trainium2 (Bass/Tile), 8-core data-parallel.

kernel(**inputs) takes the FULL unsharded inputs (as produced by
reference.setup_inputs()) and returns the full [2048] float32 output.
It shards the batch 2048 -> 8 x 256 across NeuronCores 0..7, runs one
SPMD Bass program per core (no collectives), and concatenates outputs.

Per-core layout is feature-major: activations are [feature(128 part),
batch(256 free)] tiles; batch column c = h*128 + p for half h in {0,1}.
The two batch halves run as staggered software pipelines through the
sequential GRU / AUGRU scans.

v2 design notes (vs v1):
- x-pipeline: per-step indirect gathers (Pool/SWDGE) -> PE transpose
  (fp32) -> PSUM -> Act/DVE copy-cast into a bf16 xT ring. No cast
  DMAs, no DMA transposes, nothing on the Pool queue except gathers.
- critical path folding: Wh @ h_t is computed as Wh @ p1 + (-Wh) @ m'
  (h_t = p1 - m', p1 = z*h_prev, m' = (z-1)*n), accumulated into the
  next step's PSUM so the tanh -> next-matmul hop skips the h' update.
  Same for the AUGRU r/u gates with h' = m1 - q1''.
- attention scores computed incrementally on the PE during the GRU
  (per-column matvec), softmax in SBUF, attention broadcast to all
  partitions via a rank-1 PE matmul (no DRAM round trip).
"""
import numpy as np

B_FULL, L, D = 2048, 200, 128
NCORES = 8
B = B_FULL // NCORES          # 256 per core
HB = 128                      # half-batch
ITEM_D, USER_D, CAT_D, DUR_D = 64, 64, 32, 16
UDENSE, IDENSE = 25, 3
MLP1, MLP2 = 256, 128
N_USERS, N_ITEMS, N_CAT, N_DUR = 100000, 100000, 1000, 10

CH = 8          # steps per gather-staging chunk / cast DMA
GAH = 16        # hist-gather lookahead (steps)
XP = 6          # transpose lookahead (steps)
XR = 12         # xT ring depth (steps)
SPREAD = 8      # score matvecs emitted per GRU step

# MLP concat feature groups: (name, col offset in W1, width)
GROUPS = [("user", 0, USER_D), ("item", 64, ITEM_D), ("cat", 128, CAT_D),
          ("dur", 160, DUR_D), ("ud", 176, UDENSE), ("idn", 201, IDENSE),
          ("ev", 204, D)]


def build_dien(L_steps=L, nonzero_bias=False):
    import concourse.bacc as bacc
    import concourse.mybir as mybir
    import concourse.tile as tile
    import concourse.bass as bass
    from concourse.masks import make_identity

    f32, bf16, i32 = mybir.dt.float32, mybir.dt.bfloat16, mybir.dt.int32
    AF = mybir.ActivationFunctionType
    OP = mybir.AluOpType

    nc = bacc.Bacc("TRN2", target_bir_lowering=False)

    # ---- DRAM I/O ----
    seq_d = nc.dram_tensor("history_seq", [B, L_steps], i32, kind="ExternalInput")
    uid_d = nc.dram_tensor("user_id", [B], i32, kind="ExternalInput")
    iid_d = nc.dram_tensor("item_id", [B], i32, kind="ExternalInput")
    cid_d = nc.dram_tensor("item_category", [B], i32, kind="ExternalInput")
    did_d = nc.dram_tensor("item_dur_bkt", [B], i32, kind="ExternalInput")
    ud_d = nc.dram_tensor("user_dense", [B, UDENSE], f32, kind="ExternalInput")
    id_d = nc.dram_tensor("item_dense", [B, IDENSE], f32, kind="ExternalInput")
    utab_d = nc.dram_tensor("user_table", [N_USERS, USER_D], f32, kind="ExternalInput")
    itab_d = nc.dram_tensor("item_table", [N_ITEMS, ITEM_D], f32, kind="ExternalInput")
    ctab_d = nc.dram_tensor("cat_table", [N_CAT, CAT_D], f32, kind="ExternalInput")
    dtab_d = nc.dram_tensor("dur_table", [N_DUR, DUR_D], f32, kind="ExternalInput")
    htab_d = nc.dram_tensor("hist_table", [N_ITEMS + 1, D], f32, kind="ExternalInput")
    tpw_d = nc.dram_tensor("target_proj_W", [D, ITEM_D], f32, kind="ExternalInput")
    gwi_d = nc.dram_tensor("gru_Wi", [3 * D, D], f32, kind="ExternalInput")
    gwh_d = nc.dram_tensor("gru_Wh", [3 * D, D], f32, kind="ExternalInput")
    gbi_d = nc.dram_tensor("gru_bi", [3 * D], f32, kind="ExternalInput")
    gbh_d = nc.dram_tensor("gru_bh", [3 * D], f32, kind="ExternalInput")
    awr_d = nc.dram_tensor("au_Wr", [D, 2 * D], f32, kind="ExternalInput")
    abr_d = nc.dram_tensor("au_br", [D], f32, kind="ExternalInput")
    awu_d = nc.dram_tensor("au_Wu", [D, 2 * D], f32, kind="ExternalInput")
    abu_d = nc.dram_tensor("au_bu", [D], f32, kind="ExternalInput")
    awh_d = nc.dram_tensor("au_Wh", [D, 2 * D], f32, kind="ExternalInput")
    abh_d = nc.dram_tensor("au_bh", [D], f32, kind="ExternalInput")
    w1_d = nc.dram_tensor("mlp_W1", [MLP1, 332], f32, kind="ExternalInput")
    b1_d = nc.dram_tensor("mlp_b1", [MLP1], f32, kind="ExternalInput")
    w2_d = nc.dram_tensor("mlp_W2", [MLP2, MLP1], f32, kind="ExternalInput")
    b2_d = nc.dram_tensor("mlp_b2", [MLP2], f32, kind="ExternalInput")
    w3_d = nc.dram_tensor("mlp_W3", [1, MLP2], f32, kind="ExternalInput")
    b3_d = nc.dram_tensor("mlp_b3", [1], f32, kind="ExternalInput")
    out_d = nc.dram_tensor("out", [B], f32, kind="ExternalOutput")
    attnT_dram = nc.dram_tensor("attnT_scr", [L_steps, B], bf16, kind="Internal")

    # score chunks (rows over t); keep late chunks small so most score
    # matvecs overlap the GRU scan instead of serializing after it
    tchunks = []
    c0 = 0
    while c0 < L_steps:
        rem = L_steps - c0
        if rem > 128:
            w = 128
        elif rem > 48:
            w = rem - 24
        else:
            w = rem
        tchunks.append((c0, w))
        c0 += w
    ntc = len(tchunks)

    with tile.TileContext(nc) as tc:
        import contextlib
        ctx = contextlib.ExitStack()
        with ctx:
            P = ctx.enter_context(tc.tile_pool(name="persist", bufs=1))
            WK = ctx.enter_context(tc.tile_pool(name="work", bufs=4))
            PS = ctx.enter_context(tc.tile_pool(name="psum", bufs=2, space="PSUM"))

            # ======== constants ========
            ident_bf = P.tile([128, 128], bf16)
            make_identity(nc, ident_bf[:])
            ident_f32 = P.tile([128, 128], f32)
            make_identity(nc, ident_f32[:])
            zeros_bf = P.tile([128, B], bf16)
            nc.vector.memset(zeros_bf[:], 0.0)
            ones_bf = P.tile([128, 128], bf16)
            nc.vector.memset(ones_bf[:], 1.0)

            # ======== per-batch inputs ========
            seq_sb = P.tile([128, 2, L_steps], i32)
            nc.sync.dma_start(out=seq_sb[:], in_=seq_d.rearrange("(h p) t -> p h t", h=2))
            ids_sb = P.tile([128, 4, 2], i32)
            nc.sync.dma_start(out=ids_sb[:, 0, :], in_=uid_d.rearrange("(h p) -> p h", h=2))
            nc.sync.dma_start(out=ids_sb[:, 1, :], in_=iid_d.rearrange("(h p) -> p h", h=2))
            nc.sync.dma_start(out=ids_sb[:, 2, :], in_=cid_d.rearrange("(h p) -> p h", h=2))
            nc.sync.dma_start(out=ids_sb[:, 3, :], in_=did_d.rearrange("(h p) -> p h", h=2))
            ud_sb = P.tile([128, 2, UDENSE], f32)
            nc.sync.dma_start(out=ud_sb[:], in_=ud_d.rearrange("(h p) d -> p h d", h=2))
            idn_sb = P.tile([128, 2, IDENSE], f32)
            nc.sync.dma_start(out=idn_sb[:], in_=id_d.rearrange("(h p) d -> p h d", h=2))

            # ======== x-pipeline: gather -> cast DMA -> DMA transpose ====
            hs = P.tile([128, L_steps, B], bf16)
            xT_ring = P.tile([128, XR, B], bf16)
            stg32, stg16 = {}, {}

            def issue_gather(t):
                c = t // CH
                for h in range(2):
                    if t % CH == 0:
                        stg32[(c, h)] = WK.tile([128, CH, D], f32, tag=f"g32{h}",
                                                bufs=3, name=f"g32_{c}_{h}")
                    nc.gpsimd.indirect_dma_start(
                        out=stg32[(c, h)][:, t % CH, :], out_offset=None,
                        in_=htab_d[:, :],
                        in_offset=bass.IndirectOffsetOnAxis(
                            ap=seq_sb[:, h, t:t + 1], axis=0))

            def issue_cast(c):
                for h in range(2):
                    g16 = WK.tile([128, CH, D], bf16, tag=f"g16{h}", bufs=2,
                                  name=f"g16_{c}_{h}")
                    stg16[(c, h)] = g16
                    nc.gpsimd.dma_start(out=g16[:], in_=stg32.pop((c, h))[:])

            def issue_xpose(t):
                c = t // CH
                for h in range(2):
                    nc.sync.dma_start_transpose(
                        out=xT_ring[:, t % XR, h * HB:(h + 1) * HB],
                        in_=stg16[(c, h)][:, t % CH, :])
                if t % CH == CH - 1 or t == L_steps - 1:
                    for h in range(2):
                        stg16.pop((c, h))

            # ======== weight prep ========
            def load_T(dst_bf, src_ap, rows, cols, scale=1.0):
                """dst_bf <- bf16(transpose(src_ap[rows, cols])) * scale."""
                stw = WK.tile([128, 128], f32, tag="wstg")
                nc.sync.dma_start(out=stw[:rows, :cols], in_=src_ap)
                pst = PS.tile([128, 256], f32, tag="sc", bufs=2)
                nc.tensor.transpose(pst[:cols, :rows], stw[:rows, :cols],
                                    ident_f32[:rows, :rows])
                nc.scalar.activation(out=dst_bf, in_=pst[:cols, :rows], func=AF.Copy,
                                     scale=float(scale))

            wiT = P.tile([128, 3, 128], bf16)
            whT = P.tile([128, 3, 128], bf16)
            whTn = P.tile([128, 2, 128], bf16)      # negated r,z for m' fold
            for g in range(3):
                load_T(wiT[:, g, :], gwi_d[g * 128:(g + 1) * 128, :], 128, 128)
                load_T(whT[:, g, :], gwh_d[g * 128:(g + 1) * 128, :], 128, 128)
            for g in range(2):
                load_T(whTn[:, g, :], gwh_d[g * 128:(g + 1) * 128, :], 128, 128,
                       scale=-1.0)
            auT = P.tile([128, 6, 128], bf16)   # rx, rh, ux, uh, hx, hh
            auTn = P.tile([128, 2, 128], bf16)  # negated rh, uh for q1'' fold
            for gi, wd in enumerate((awr_d, awu_d, awh_d)):
                load_T(auT[:, 2 * gi + 0, :], wd[:, 0:128], 128, 128)
                load_T(auT[:, 2 * gi + 1, :], wd[:, 128:256], 128, 128)
            for gi, wd in enumerate((awr_d, awu_d)):
                load_T(auTn[:, gi, :], wd[:, 128:256], 128, 128, scale=-1.0)
            tpT = P.tile([ITEM_D, 128], bf16)
            load_T(tpT[:], tpw_d[:, :], D, ITEM_D, scale=1.0 / float(np.sqrt(D)))
            w1T = {}
            for (gname, off, w) in GROUPS:
                tl = P.tile([w, 2, 128], bf16, tag=f"w1T_{gname}", name=f"w1T_{gname}")
                for m in range(2):
                    load_T(tl[:w, m, :], w1_d[m * 128:(m + 1) * 128, off:off + w], 128, w)
                w1T[gname] = tl
            w2T = P.tile([128, 2, 128], bf16)
            for m in range(2):
                load_T(w2T[:, m, :], w2_d[:, m * 128:(m + 1) * 128], 128, 128)
            w3T = P.tile([128, 1], bf16)
            load_T(w3T[:], w3_d[:, :], 1, 128)

            # biases (all-zero in this model; nonzero handled via rank-1 mms
            # with row-transposed biases + activation bias APs)
            gbi_sb = P.tile([128, 3], f32)
            nc.sync.dma_start(out=gbi_sb[:], in_=gbi_d.rearrange("(g p) -> p g", g=3))
            aub_sb = P.tile([128, 3], f32)
            nc.sync.dma_start(out=aub_sb[:, 0:1], in_=abr_d.rearrange("(a p) -> p a", a=1))
            nc.sync.dma_start(out=aub_sb[:, 1:2], in_=abu_d.rearrange("(a p) -> p a", a=1))
            nc.sync.dma_start(out=aub_sb[:, 2:3], in_=abh_d.rearrange("(a p) -> p a", a=1))
            b1_sb = P.tile([128, 2], f32)
            nc.sync.dma_start(out=b1_sb[:], in_=b1_d.rearrange("(m p) -> p m", m=2))
            b2_sb = P.tile([128, 1], f32)
            nc.sync.dma_start(out=b2_sb[:], in_=b2_d.rearrange("(a p) -> p a", a=1))
            b3_sb = P.tile([1, 1], f32)
            nc.sync.dma_start(out=b3_sb[:], in_=b3_d.rearrange("(a p) -> p a", a=1))
            if nonzero_bias:
                gbh_sb = P.tile([128, 3], f32)
                nc.sync.dma_start(out=gbh_sb[:], in_=gbh_d.rearrange("(g p) -> p g", g=3))
                grz_sum = P.tile([128, 3], f32)
                nc.vector.tensor_tensor(out=grz_sum[:], in0=gbi_sb[:], in1=gbh_sb[:],
                                        op=OP.add)
                # transpose bias columns to rows (for rank-1 bias matmuls)
                brow = P.tile([4, 128], bf16)   # rows: gru r+z sums, au r, au u
                bst = WK.tile([128, 4], f32, tag="wstg2")
                nc.vector.tensor_copy(out=bst[:, 0:2], in_=grz_sum[:, 0:2])
                nc.vector.tensor_copy(out=bst[:, 2:3], in_=aub_sb[:, 0:1])
                nc.vector.tensor_copy(out=bst[:, 3:4], in_=aub_sb[:, 1:2])
                pstb = PS.tile([128, 256], f32, tag="sc", bufs=2)
                nc.tensor.transpose(pstb[:4, :128], bst[:, :], ident_f32[:])
                nc.scalar.activation(out=brow[:], in_=pstb[:4, :128], func=AF.Copy)
                ones_row = P.tile([1, B], bf16)
                nc.vector.memset(ones_row[:], 1.0)

            # ======== small embedding gathers (feature-major) ========
            grp_sb = {}
            for (gname_, _, w_) in GROUPS:
                if gname_ != "ev":
                    grp_sb[gname_] = P.tile([w_, B], bf16, tag=f"xg_{gname_}",
                                            name=f"xg_{gname_}")

            def gather_T(tab_ap, idx_k, width, dst):
                for h in range(2):
                    g = WK.tile([128, width], f32, tag=f"g{width}", name=f"g{width}_{h}")
                    nc.gpsimd.indirect_dma_start(
                        out=g[:], out_offset=None, in_=tab_ap,
                        in_offset=bass.IndirectOffsetOnAxis(
                            ap=ids_sb[:, idx_k, h:h + 1], axis=0))
                    pst2 = PS.tile([128, 256], f32, tag="sc", bufs=2)
                    nc.tensor.transpose(pst2[:width, :128], g[:], ident_f32[:])
                    nc.scalar.activation(out=dst[:, h * HB:(h + 1) * HB],
                                         in_=pst2[:width, :128], func=AF.Copy)

            # ======== pre-loop: fill gather/transpose pipeline ========
            for t in range(min(GAH, L_steps)):
                issue_gather(t)
            import math
            for c in range(min(GAH // CH, math.ceil(L_steps / CH))):
                issue_cast(c)
            # small gathers after the first hist gathers (Pool queue order)
            gather_T(utab_d[:, :], 0, USER_D, grp_sb["user"])
            gather_T(itab_d[:, :], 1, ITEM_D, grp_sb["item"])
            gather_T(ctab_d[:, :], 2, CAT_D, grp_sb["cat"])
            gather_T(dtab_d[:, :], 3, DUR_D, grp_sb["dur"])
            for h in range(2):
                pst3 = PS.tile([128, 256], f32, tag="sc", bufs=2)
                nc.tensor.transpose(pst3[:UDENSE, :128], ud_sb[:, h, :], ident_f32[:])
                nc.scalar.activation(out=grp_sb["ud"][:, h * HB:(h + 1) * HB],
                                     in_=pst3[:UDENSE, :128], func=AF.Copy)
                pst4 = PS.tile([128, 256], f32, tag="sc", bufs=2)
                nc.tensor.transpose(pst4[:IDENSE, :128], idn_sb[:, h, :], ident_f32[:])
                nc.scalar.activation(out=grp_sb["idn"][:, h * HB:(h + 1) * HB],
                                     in_=pst4[:IDENSE, :128], func=AF.Copy)

            # target^T = (1/sqrt(D)) * W_p @ item_emb^T : [D, B] bf16
            tgt_ps = PS.tile([128, 256], f32, tag="sc", bufs=2)
            nc.tensor.matmul(tgt_ps[:, :], tpT[:], grp_sb["item"][:],
                             start=True, stop=True, skip_group_check=True)
            tgt_bf = P.tile([128, B], bf16)
            nc.vector.tensor_copy(out=tgt_bf[:], in_=tgt_ps[:, :])

            for t in range(min(XP, L_steps)):
                issue_xpose(t)

            # ======== GRU state ========
            rz_ps, nxh_ps = {}, {}

            def issue_wi_rz(t):
                p = PS.tile([128, 512], f32, tag="rz", bufs=3, name=f"rz_{t}")
                rz_ps[t] = p
                xt = xT_ring[:, t % XR, :]
                last = (t == 0)
                nc.tensor.matmul(p[:, 0:256], wiT[:, 0, :], xt,
                                 start=True, stop=last, skip_group_check=True)
                nc.tensor.matmul(p[:, 256:512], wiT[:, 1, :], xt,
                                 start=True, stop=last, skip_group_check=True)
                if nonzero_bias:
                    nc.tensor.matmul(p[:, 0:256], brow[0:1, :], ones_row[:],
                                     start=False, stop=False, skip_group_check=True)
                    nc.tensor.matmul(p[:, 256:512], brow[1:2, :], ones_row[:],
                                     start=False, stop=False, skip_group_check=True)

            def issue_wi_n(t):
                p = PS.tile([128, 512], f32, tag="nxh", bufs=2, name=f"nxh_{t}")
                nxh_ps[t] = p
                xt = xT_ring[:, t % XR, :]
                nc.tensor.matmul(p[:, 0:256], wiT[:, 2, :], xt,
                                 start=True, stop=True, skip_group_check=True)

            gmid = {}

            def gru_A(t, h):
                """sigma + p1 + fold mms + t_, v for (step t, half h)."""
                cb = slice(h * HB, h * HB + HB)
                rzp = rz_ps[t]
                rz4 = rzp[:].rearrange("p (a b) -> p a b", b=128)
                rz_sb = WK.tile([128, 256], bf16, tag=f"rz{h}", bufs=2,
                                name=f"rz_{t}_{h}")
                nc.scalar.activation(out=rz_sb[:].rearrange("p (a b) -> p a b", b=128),
                                     in_=rz4[:, h::2, :], func=AF.Sigmoid)
                if t > 0:
                    hp = hs[:, t - 1, cb]
                    tv = WK.tile([128, 128], bf16, tag=f"t{h}", bufs=2,
                                 name=f"t_{t}_{h}")
                    nc.vector.tensor_tensor(
                        out=tv[:], in0=rz_sb[:, 0:128],
                        in1=nxh_ps[t][:, 256 + h * HB:256 + h * HB + HB], op=OP.mult)
                    v = WK.tile([128, 128], bf16, tag=f"v{h}", bufs=2,
                                name=f"v_{t}_{h}")
                    nc.vector.tensor_tensor(
                        out=v[:], in0=tv[:],
                        in1=nxh_ps[t][:, h * HB:h * HB + HB], op=OP.add)
                    p1 = WK.tile([128, 128], bf16, tag=f"p1{h}", bufs=2,
                                 name=f"p1_{t}_{h}")
                    nc.vector.tensor_tensor(out=p1[:], in0=rz_sb[:, 128:256], in1=hp,
                                            op=OP.mult)
                    if t + 1 < L_steps:
                        nxt = rz_ps[t + 1]
                        nc.tensor.matmul(nxt[:, h * HB:h * HB + HB],
                                         whT[:, 0, :], p1[:],
                                         start=False, stop=False, skip_group_check=True)
                        nc.tensor.matmul(nxt[:, 256 + h * HB:256 + h * HB + HB],
                                         whT[:, 1, :], p1[:],
                                         start=False, stop=False, skip_group_check=True)
                else:
                    p1, v = None, None
                gmid[(t, h)] = (rz_sb, p1, v)

            def gru_B(t, h):
                """tanh + m' + fold mms + h' for (step t, half h)."""
                cb = slice(h * HB, h * HB + HB)
                rz_sb, p1, v = gmid.pop((t, h))
                n = WK.tile([128, 128], bf16, tag=f"n{h}", bufs=2, name=f"n_{t}_{h}")
                if t > 0:
                    nc.scalar.activation(out=n[:], in_=v[:], func=AF.Tanh,
                                         bias=gbi_sb[:, 2:3] if nonzero_bias else 0.0)
                else:
                    nc.scalar.activation(out=n[:],
                                         in_=nxh_ps[t][:, h * HB:h * HB + HB],
                                         func=AF.Tanh,
                                         bias=gbi_sb[:, 2:3] if nonzero_bias else 0.0)
                mp = WK.tile([128, 128], bf16, tag=f"m{h}", bufs=2, name=f"m_{t}_{h}")
                nc.vector.scalar_tensor_tensor(out=mp[:], in0=rz_sb[:, 128:256],
                                               scalar=1.0, in1=n[:],
                                               op0=OP.subtract, op1=OP.mult)
                if t + 1 < L_steps:
                    nxt = rz_ps[t + 1]
                    nc.tensor.matmul(nxt[:, h * HB:h * HB + HB], whTn[:, 0, :], mp[:],
                                     start=False, stop=True, skip_group_check=True)
                    nc.tensor.matmul(nxt[:, 256 + h * HB:256 + h * HB + HB],
                                     whTn[:, 1, :], mp[:],
                                     start=False, stop=True, skip_group_check=True)
                if t > 0:
                    nc.vector.tensor_tensor(out=hs[:, t, cb], in0=p1[:], in1=mp[:],
                                            op=OP.subtract)
                else:
                    nc.vector.tensor_tensor(out=hs[:, t, cb], in0=zeros_bf[:, 0:HB],
                                            in1=mp[:], op=OP.subtract)
                if t + 1 < L_steps:
                    nc.tensor.matmul(nxh_ps[t + 1][:, 256 + h * HB:256 + h * HB + HB],
                                     whT[:, 2, :], hs[:, t, cb],
                                     start=True, stop=True, skip_group_check=True)

            # ======== incremental attention scores ========
            sc_sb = P.tile([128, ntc, B], f32)
            sc_ps = {}
            score_jobs = []      # pending (ci, b) matvecs
            chunk_left = {}

            def chunk_ready(t):
                for ci, (c0_, rows) in enumerate(tchunks):
                    if c0_ + rows - 1 == t - 1:
                        p = PS.tile([128, 256], f32, tag="sc", bufs=2,
                                    name=f"scps_{ci}")
                        sc_ps[ci] = p
                        for b in range(B):
                            score_jobs.append((ci, b))
                        chunk_left[ci] = B

            def emit_scores(k):
                while k > 0 and score_jobs:
                    ci, b = score_jobs.pop(0)
                    c0_, rows = tchunks[ci]
                    nc.tensor.matmul(sc_ps[ci][:rows, b:b + 1],
                                     hs[:, c0_:c0_ + rows, b:b + 1],
                                     tgt_bf[:, b:b + 1],
                                     start=True, stop=True, skip_group_check=True)
                    chunk_left[ci] -= 1
                    if chunk_left[ci] == 0:
                        nc.vector.tensor_copy(out=sc_sb[:rows, ci, :],
                                              in_=sc_ps.pop(ci)[:rows, :])
                    k -= 1

            # ======== GRU scan ========
            issue_wi_rz(0)
            issue_wi_n(0)
            issue_wi_rz(1)
            issue_wi_n(1)
            gru_A(0, 0)
            for t in range(L_steps):
                tg = t + GAH
                if tg < L_steps:
                    issue_gather(tg)
                    if tg == L_steps - 1 or tg % CH == CH - 1:
                        issue_cast(tg // CH)
                if t + XP < L_steps:
                    issue_xpose(t + XP)
                if t > 0:
                    gru_B(t - 1, 1)
                if t + 2 < L_steps:
                    issue_wi_rz(t + 2)
                if 2 <= t + 1 < L_steps:
                    issue_wi_n(t + 1)
                gru_B(t, 0)
                gru_A(t, 1)
                if t + 1 < L_steps:
                    gru_A(t + 1, 0)
                chunk_ready(t)
                emit_scores(SPREAD)
            gru_B(L_steps - 1, 1)
            chunk_ready(L_steps)
            emit_scores(len(score_jobs))

            # ======== softmax (b-major) -> attnT (t-major, bf16) ========
            attnT_sb = P.tile([128, ntc, B], bf16)
            for h in range(2):
                scb = WK.tile([128, L_steps], f32, tag="scb", bufs=2, name=f"scb{h}")
                for ci, (c0_, rows) in enumerate(tchunks):
                    pst5 = PS.tile([128, 256], f32, tag="xp", bufs=1)
                    nc.tensor.transpose(pst5[:128, :rows],
                                        sc_sb[:rows, ci, h * HB:(h + 1) * HB],
                                        ident_f32[:rows, :rows])
                    nc.vector.tensor_copy(out=scb[:, c0_:c0_ + rows],
                                          in_=pst5[:128, :rows])
                m01 = WK.tile([128, L_steps], f32, tag="m01")
                nc.vector.tensor_scalar(out=m01[:], in0=seq_sb[:, h, :], scalar1=0,
                                        scalar2=None, op0=OP.is_gt)
                sm = WK.tile([128, L_steps], f32, tag="sm")
                nc.vector.scalar_tensor_tensor(out=sm[:], in0=scb[:], scalar=1e9,
                                               in1=m01[:], op0=OP.add, op1=OP.mult)
                rmax = WK.tile([128, 1], f32, tag="rmax")
                nc.vector.tensor_reduce(out=rmax[:], in_=sm[:],
                                        axis=mybir.AxisListType.X, op=OP.max, negate=True)
                ex = WK.tile([128, L_steps], f32, tag="ex")
                nc.scalar.activation(out=ex[:], in_=sm[:], func=AF.Exp, bias=rmax[:])
                rsum = WK.tile([128, 1], f32, tag="rsum")
                nc.vector.tensor_reduce(out=rsum[:], in_=ex[:],
                                        axis=mybir.AxisListType.X, op=OP.add)
                rinv = WK.tile([128, 1], f32, tag="rinv")
                nc.vector.reciprocal(out=rinv[:], in_=rsum[:])
                attn_b = WK.tile([128, L_steps], f32, tag="attnb", bufs=2,
                                 name=f"attnb{h}")
                nc.vector.tensor_scalar(out=attn_b[:], in0=ex[:], scalar1=rinv[:],
                                        scalar2=None, op0=OP.mult)
                for ci, (c0_, rows) in enumerate(tchunks):
                    pst6 = PS.tile([128, 256], f32, tag="sc", bufs=2)
                    nc.tensor.transpose(pst6[:rows, :128], attn_b[:, c0_:c0_ + rows],
                                        ident_f32[:])
                    nc.vector.tensor_copy(out=attnT_sb[:rows, ci, h * HB:(h + 1) * HB],
                                          in_=pst6[:rows, :128])
            for ci, (c0_, rows) in enumerate(tchunks):
                nc.sync.dma_start(out=attnT_dram[c0_:c0_ + rows, :],
                                  in_=attnT_sb[:rows, ci, :])

            # ======== AUGRU scan ========
            ah = P.tile([128, 2, B], bf16)
            ARING, ABC = 12, 4
            a_ring = P.tile([128, ARING, B], bf16)
            ru_ps, n_ps = {}, {}

            def issue_abcast(t0):
                kw = min(ABC, L_steps - t0)
                src = attnT_dram[t0:t0 + kw, :]
                src_b = bass.AP(tensor=src.tensor, offset=src.offset,
                                ap=[[0, 128]] + list(src.ap))
                nc.gpsimd.dma_start(out=a_ring[:, t0 % ARING:t0 % ARING + kw, :],
                                    in_=src_b)

            def a_of(t, h):
                return a_ring[:, t % ARING, h * HB:h * HB + HB]

            def issue_aux(t):
                p = PS.tile([128, 512], f32, tag="rz", bufs=3, name=f"ru_{t}")
                ru_ps[t] = p
                x = hs[:, t, :]
                last = (t == 0)
                nc.tensor.matmul(p[:, 0:256], auT[:, 0, :], x,
                                 start=True, stop=last, skip_group_check=True)
                nc.tensor.matmul(p[:, 256:512], auT[:, 2, :], x,
                                 start=True, stop=last, skip_group_check=True)
                if nonzero_bias:
                    nc.tensor.matmul(p[:, 0:256], brow[2:3, :], ones_row[:],
                                     start=False, stop=False, skip_group_check=True)
                    nc.tensor.matmul(p[:, 256:512], brow[3:4, :], ones_row[:],
                                     start=False, stop=False, skip_group_check=True)

            def issue_aux_n(t):
                p = PS.tile([128, 512], f32, tag="nxh", bufs=2, name=f"nau_{t}")
                n_ps[t] = p
                nc.tensor.matmul(p[:, 0:256], auT[:, 4, :], hs[:, t, :],
                                 start=True, stop=(t == 0), skip_group_check=True)

            amid = {}

            def au_A(t, h):
                cb = slice(h * HB, h * HB + HB)
                rup = ru_ps[t]
                ru4 = rup[:].rearrange("p (a b) -> p a b", b=128)
                ru_sb = WK.tile([128, 256], bf16, tag=f"ru{h}", bufs=2,
                                name=f"ru_{t}_{h}")
                nc.scalar.activation(out=ru_sb[:].rearrange("p (a b) -> p a b", b=128),
                                     in_=ru4[:, h::2, :], func=AF.Sigmoid)
                if t > 0:
                    hp = ah[:, (t - 1) % 2, cb]
                    rh = WK.tile([128, 128], bf16, tag=f"rh{h}", bufs=2,
                                 name=f"rh_{t}_{h}")
                    nc.vector.tensor_tensor(out=rh[:], in0=ru_sb[:, 0:128], in1=hp,
                                            op=OP.mult)
                    nc.tensor.matmul(n_ps[t][:, cb], auT[:, 5, :], rh[:],
                                     start=False, stop=True, skip_group_check=True)
                w = WK.tile([128, 128], bf16, tag=f"w{h}", bufs=2, name=f"w_{t}_{h}")
                nc.vector.tensor_tensor(out=w[:], in0=ru_sb[:, 128:256],
                                        in1=a_of(t, h), op=OP.mult)
                if t > 0:
                    q1 = WK.tile([128, 128], bf16, tag=f"q1{h}", bufs=2,
                                 name=f"q1_{t}_{h}")
                    nc.vector.scalar_tensor_tensor(out=q1[:], in0=w[:], scalar=1.0,
                                                   in1=hp, op0=OP.subtract, op1=OP.mult)
                    if t + 1 < L_steps:
                        nxt = ru_ps[t + 1]
                        nc.tensor.matmul(nxt[:, h * HB:h * HB + HB],
                                         auTn[:, 0, :], q1[:],
                                         start=False, stop=False, skip_group_check=True)
                        nc.tensor.matmul(nxt[:, 256 + h * HB:256 + h * HB + HB],
                                         auTn[:, 1, :], q1[:],
                                         start=False, stop=False, skip_group_check=True)
                else:
                    q1 = None
                amid[(t, h)] = (w, q1)

            def au_B(t, h):
                cb = slice(h * HB, h * HB + HB)
                w, q1 = amid.pop((t, h))
                n2 = WK.tile([128, 128], bf16, tag=f"n2{h}", bufs=2, name=f"n2_{t}_{h}")
                nc.scalar.activation(out=n2[:], in_=n_ps[t][:, cb], func=AF.Tanh,
                                     bias=aub_sb[:, 2:3] if nonzero_bias else 0.0)
                m1 = WK.tile([128, 128], bf16, tag=f"m1{h}", bufs=2, name=f"m1_{t}_{h}")
                nc.vector.tensor_tensor(out=m1[:], in0=w[:], in1=n2[:], op=OP.mult)
                if t + 1 < L_steps:
                    nxt = ru_ps[t + 1]
                    nc.tensor.matmul(nxt[:, h * HB:h * HB + HB], auT[:, 1, :], m1[:],
                                     start=False, stop=True, skip_group_check=True)
                    nc.tensor.matmul(nxt[:, 256 + h * HB:256 + h * HB + HB],
                                     auT[:, 3, :], m1[:],
                                     start=False, stop=True, skip_group_check=True)
                if t > 0:
                    nc.vector.tensor_tensor(out=ah[:, t % 2, cb], in0=m1[:], in1=q1[:],
                                            op=OP.subtract)
                else:
                    nc.vector.tensor_tensor(out=ah[:, t % 2, cb], in0=m1[:],
                                            in1=zeros_bf[:, 0:HB], op=OP.subtract)

            for t0 in range(0, min(ARING, L_steps), ABC):
                issue_abcast(t0)
            issue_aux(0)
            issue_aux_n(0)
            issue_aux(1)
            issue_aux_n(1)
            au_A(0, 0)
            for t in range(L_steps):
                if t > 0:
                    au_B(t - 1, 1)
                if t + 2 < L_steps:
                    issue_aux(t + 2)
                if 2 <= t + 1 < L_steps:
                    issue_aux_n(t + 1)
                au_B(t, 0)
                au_A(t, 1)
                if t + 1 < L_steps:
                    au_A(t + 1, 0)
                tf = t + ARING // 2
                if tf < L_steps and tf % ABC == 0:
                    issue_abcast(tf)
            au_B(L_steps - 1, 1)

            evolved = ah[:, (L_steps - 1) % 2, :]
            grp_rhs = dict(grp_sb)
            grp_rhs["ev"] = evolved

            # ======== MLP head ========
            h1_sb = P.tile([128, 2, B], bf16)
            for m in range(2):
                h1_ps = PS.tile([128, 256], f32, tag="xp", bufs=1)
                for gi, (gname, off, w) in enumerate(GROUPS):
                    nc.tensor.matmul(h1_ps[:, :], w1T[gname][:w, m, :],
                                     grp_rhs[gname][:] if gname == "ev" else grp_rhs[gname][:w, :],
                                     start=(gi == 0), stop=(gi == len(GROUPS) - 1),
                                     skip_group_check=True)
                nc.scalar.activation(out=h1_sb[:, m, :], in_=h1_ps[:, :], func=AF.Relu,
                                     bias=b1_sb[:, m:m + 1])
            h2_ps = PS.tile([128, 256], f32, tag="sc", bufs=2)
            nc.tensor.matmul(h2_ps[:, :], w2T[:, 0, :], h1_sb[:, 0, :],
                             start=True, stop=False, skip_group_check=True)
            nc.tensor.matmul(h2_ps[:, :], w2T[:, 1, :], h1_sb[:, 1, :],
                             start=False, stop=True, skip_group_check=True)
            h2_sb = P.tile([128, B], bf16)
            nc.scalar.activation(out=h2_sb[:], in_=h2_ps[:, :], func=AF.Relu,
                                 bias=b2_sb[:])
            lg_ps = PS.tile([128, 256], f32, tag="xp", bufs=1)
            nc.tensor.matmul(lg_ps[0:1, :], w3T[:], h2_sb[:],
                             start=True, stop=True, skip_group_check=True)
            out_sb = P.tile([1, B], f32)
            nc.scalar.activation(out=out_sb[:], in_=lg_ps[0:1, :], func=AF.Sigmoid,
                                 bias=b3_sb[0:1, :])
            nc.sync.dma_start(out=out_d.rearrange("(a b) -> a b", a=1), in_=out_sb[:])

    nc.finalize()
    return nc


_NC_CACHE = {}


def kernel(**inputs):
    from concourse import bass_utils

    inputs = {k: np.asarray(v) for k, v in inputs.items()}
    L_steps = inputs["history_seq"].shape[1]
    bias_names = ["gru_bi", "gru_bh", "au_br", "au_bu", "au_bh"]
    nonzero_bias = any(np.any(inputs[k]) for k in bias_names)

    key = (L_steps, nonzero_bias)
    if key not in _NC_CACHE:
        _NC_CACHE[key] = build_dien(L_steps, nonzero_bias)
    nc = _NC_CACHE[key]

    per_b = ["user_id", "item_id", "item_category", "item_dur_bkt",
             "history_seq", "user_dense", "item_dense"]
    shared = {k: np.ascontiguousarray(v) for k, v in inputs.items() if k not in per_b}
    in_maps = []
    for c in range(NCORES):
        m = dict(shared)
        for k in per_b:
            m[k] = np.ascontiguousarray(inputs[k][c * B:(c + 1) * B])
        in_maps.append(m)

    res = bass_utils.run_bass_kernel_spmd(nc, in_maps, core_ids=list(range(NCORES)))
    out = np.concatenate([res.results[c]["out"] for c in range(NCORES)])
    return out.astype(np.float32)


if __name__ == "__main__":
    import os
    import importlib.util
    spec = importlib.util.spec_from_file_location("reference", "/root/problem/reference.py")
    ref = importlib.util.module_from_spec(spec)
    spec.loader.exec_module(ref)
    ins = {k: np.asarray(v) for k, v in ref.setup_inputs().items()}
    Lt = int(os.environ.get("DIEN_L", "8"))
    if Lt < L:
        ins["history_seq"] = np.ascontiguousarray(ins["history_seq"][:, :Lt])
    import jax.numpy as jnp
    exp = np.asarray(ref.reference(**{k: jnp.asarray(v) for k, v in ins.items()}))
    got = kernel(**ins)
    err = np.abs(got - exp)
    rel = err / np.maximum(np.abs(exp), 1e-6)
    print(f"L={Lt} max_abs={err.max():.3e} max_rel={rel.max():.3e} mean_rel={rel.mean():.3e}")
